# revision 42
# baseline (speedup 1.0000x reference)
"""Causal multi-head attention for Trainium2, 8-core (batch x head-half) parallel.

Problem: B=4, S=2048, D=1024, H=16 heads (dk=64), fp32 in/out.
    q = x @ w_q.T ; k = x @ w_k.T ; v = x @ w_v.T   (per-head split)
    out = softmax(causal(q k^T / 8)) v, concat heads, @ w_o.T + b_o

Sharding: core c owns batch b = c//2 and head-half hh = c%2 (8 heads =
channels [512*hh, 512*hh+512)).  Each core computes q/k/v projections for
its 512 channels over its one batch, runs causal attention for its 8 heads,
and produces a partial output projection outT_c = w_o[:, ch]^T a_c^T of
shape [1024, S]; the host sums core pairs (2b, 2b+1), transposes, adds b_o.

All matmul operands are bf16 (fp32 PSUM accumulation; validated 3.8e-3 max
rel err in numpy vs the 2e-2 gate).  bf16 keeps the PE at 1 cycle/row,
enables FWL fast weight loads, and halves SBUF/DMA vs fp32.

Per-core dataflow (head-pair t = 0..3 maps to SBUF partition tiles):
  - x is pre-transposed + bf16 on host: xT [8, 128, S] so the contraction
    dim D lands on SBUF partitions.
  - projections run weight-stationary kt-outer: one LDWEIGHTS feeds 4
    matmuls (one per 512-token chunk), accumulating in 4 PSUM banks.
  - v is PE-transposed to natural [tok, ch] order and stored as
    [v_h | ones] stationaries: AV then yields both the attention output
    (rows 0-63) and the softmax denominator replicated on rows 64-127.
  - scores are computed transposed (keys on partitions): sT = kT^T qT with
    two heads running concurrently in PE row groups 0-1 / 2-3.
  - softmax without max-subtraction (scores ~N(0,1); exp in fp32 PSUM),
    causal handled by an additive -1e9 triangle mask on exact-diagonal
    128x128 blocks; above-diagonal work inside a diagonal 512-superblock is
    skipped by trimming the matmul free dim (queries < 128*i are never
    computed or exp'd, and the AV accumulation never reads them).
  - normalization: numerators/denominators are evacuated copy-first into
    per-head-pair wide tiles (frees PSUM fast); ONE batched Ln + Exp
    (1/x = exp(-ln x), both functions in one ACT table set) + one DVE mul
    per head-pair replaces 8 small ACT calls (ACT is the attention-phase
    bottleneck).  The last head-pair normalizes per chunk instead so only
    the final chunk's small Ln/Exp separates attention from the output
    projection.
  - attention is a single rolling software pipeline over all (head-pair,
    chunk, key-tile) steps with LA=2 score-tiles of lookahead.  Deferred
    work fills the PE whenever the window is ACT-bound: head-pair t's
    q/k/v projections run one kt-slice per step of head-pair t-1's
    attention on a dedicated 1-slot "proj" PSUM tag (t0's chunks 2-3
    fill t0's own thin ramp-in); the first 4 output-projection groups'
    ct=0..2 partials run inside the last head-pair's window and spill to
    SBUF bf16.  PSUM: 2x2-bank score slots + 1x2-bank proj slot +
    2x1-bank AV accumulators = exactly 8 banks.
  - a post-pass NoOp-ifies LDWEIGHTS instructions identical to their
    predecessor (bass re-emits one per matmul; each costs ~95ns of dead
    PE time since FWL/background weight buffering is absent), and the
    output projection runs a 3-deep software pipeline of (row-tile,
    chunk-pair) groups with bf16 outputs DMA'd per group (host sums the
    two per-batch partials in fp32).
  - measured (neuron-profile over 8 cores): ~321.6 us max / ~316 us mean
    vs 334.7 us for the previous all-upfront schedule (7.06 ms for the
    original fp32 head-sharded baseline); rel err 4.4e-3 vs the 2e-2
    gate.  fp8 DoubleRow was evaluated and rejected: e4m3 operand
    quantization exceeds the accuracy gate (8e-2 on projections alone).
"""

import numpy as np

import concourse.bass as bass
import concourse.tile as tile
from concourse import mybir
from concourse import bass_utils

f32 = mybir.dt.float32
f32r = mybir.dt.float32r
bf16 = mybir.dt.bfloat16
fp16 = mybir.dt.float16
u32 = mybir.dt.uint32
AF = mybir.ActivationFunctionType

B, S, D, H = 4, 2048, 1024, 16
DK = D // H            # 64
NCORES = 8
PT = 128               # partition tile
CH = 512               # query chunk (PSUM bank = 512 fp32)
KT = D // PT           # 8 contraction tiles over D
T = 4                  # head-pairs per core (8 heads)
NT = D // PT           # 8 output row tiles for the o-projection
NEG = -1.0e9


def _split_multi_waits(nc):
    """This walrus build allows at most one sync-wait per TPB instruction;
    hoist extra waits onto single-wait NoOps on the same engine."""
    n = 0
    for f in nc.m.functions:
        for blk in f.blocks:
            new = []
            for inst in blk.instructions:
                si = inst.sync_info
                if si is not None and si.on_wait and len(si.on_wait) > 1:
                    ws = list(si.on_wait)
                    for w in ws[:-1]:
                        new.append(mybir.InstNoOp(
                            name=f"I-wfix-{n}", ins=[], outs=[], engine=inst.engine,
                            sync_info=mybir.SyncInfo(on_wait=[w], on_update=[])))
                        n += 1
                    inst.sync_info = mybir.SyncInfo(
                        on_wait=[ws[-1]], on_update=list(si.on_update))
                new.append(inst)
            blk.instructions = new
    return n


def _dedup_ldweights(nc):
    """The PE pays ~95ns of dead time per LDWEIGHTS (no FWL / background
    buffer in this build), and bass re-emits an identical LDWEIGHTS for
    every matmul even when consecutive matmuls share a stationary (e.g.
    the two chunk-matmuls of one projection kt-slice).  Replace a
    Tensor-queue LDWEIGHTS whose operand is byte-identical to the
    immediately-preceding one (with only matmuls/noops/semaphores in
    between, and those matmuls not in transpose mode) by a NoOp carrying
    the same sync_info.  All stationary tiles here are write-once, so an
    identical AP means identical weights."""
    n = 0
    for f in nc.m.functions:
        for blk in f.blocks:
            last_sig = None
            pe_eng = None
            new = []
            for inst in blk.instructions:
                if isinstance(inst, mybir.InstLdweights):
                    pe_eng = inst.engine
                    sig = (str(inst.ins[0]),
                           bool(inst.is_transpose), str(inst.perf_mode),
                           str(inst.tile_position))
                    if sig == last_sig:
                        si = inst.sync_info
                        if si is not None and (si.on_wait or si.on_update):
                            new.append(mybir.InstNoOp(
                                name=f"I-lwdup-{n}", ins=[], outs=[],
                                engine=inst.engine, sync_info=si))
                        n += 1
                        continue
                    last_sig = sig
                elif inst.engine != pe_eng:
                    # other engines don't touch the PE weight registers (all
                    # stationary tiles in this kernel are write-once before
                    # first use, so cross-engine writes can't invalidate)
                    pass
                elif isinstance(inst, (mybir.InstMatmult, mybir.InstNoOp,
                                       mybir.InstEventSemaphore)):
                    if getattr(inst, "is_transpose", False):
                        last_sig = None
                else:
                    last_sig = None
                new.append(inst)
            blk.instructions = new
    return n


def build(Sc=S, split_waits=True, p_bufs=6, vt_bufs=2, dm_bufs=2, nm_bufs=3,
          os_bufs=4, acc_bufs=2, att_bufs=2, do_attn=True, do_outproj=True):
    """Build the per-core Bass program. Same program for all 8 cores; only
    the input data differs per core."""
    from contextlib import ExitStack

    NCH = Sc // CH         # query chunks
    NTT = Sc // PT         # token/key tiles

    nc = bass.Bass("TRN2", target_bir_lowering=False, debug=False)

    xT_d = nc.dram_tensor("xT", [KT, PT, Sc], bf16, kind="ExternalInput")
    wq_d = nc.dram_tensor("wq", [PT, KT, T, PT], bf16, kind="ExternalInput")
    wk_d = nc.dram_tensor("wk", [PT, KT, T, PT], bf16, kind="ExternalInput")
    wv_d = nc.dram_tensor("wv", [PT, KT, T, PT], bf16, kind="ExternalInput")
    wo_d = nc.dram_tensor("wo", [PT, T, NT, PT], bf16, kind="ExternalInput")
    id_d = nc.dram_tensor("ident", [PT, PT], bf16, kind="ExternalInput")
    mask_d = nc.dram_tensor("mask", [PT, PT], f32, kind="ExternalInput")
    # bf16 partial outputs: halves the 8MB/core output DMA (host sums the
    # two per-batch partials in fp32; bf16 rounding adds ~4e-4 to the gate)
    out_d = nc.dram_tensor("outT", [D, Sc], bf16, kind="ExternalOutput")

    with tile.TileContext(nc) as tc, ExitStack() as ctx:
        singles = ctx.enter_context(tc.tile_pool(name="singles", bufs=1))
        # phase-limited tensors share slots: wq/wk/wv (phase 1) and wo
        # (phase 3) rotate through 3 slots; the 8 x tiles (phase 1) and the
        # 4 aT tiles (phases 2-3) rotate through 8 slots.
        pool_w = ctx.enter_context(tc.tile_pool(name="w", bufs=3))
        pool_xa = ctx.enter_context(tc.tile_pool(name="xa", bufs=8))
        pool_P = ctx.enter_context(tc.tile_pool(name="P", bufs=p_bufs))
        pool_vt = ctx.enter_context(tc.tile_pool(name="vt", bufs=vt_bufs))
        # per-head-pair wide normalization tiles: denominators and numerators
        # for all NCH chunks are collected, then ONE Ln + ONE Exp + ONE mul
        # per head-pair replaces the 4-per-chunk ACT calls (the ACT engine is
        # the attention-phase bottleneck; this removes ~30us of ACT work and
        # keeps the small norm calls out of the critical exp stream).
        pool_dn = ctx.enter_context(tc.tile_pool(name="dn", bufs=2))
        pool_nmw = ctx.enter_context(tc.tile_pool(name="nmw", bufs=2))
        pool_dmw = ctx.enter_context(tc.tile_pool(name="dmw", bufs=2))
        pool_os = ctx.enter_context(tc.tile_pool(name="os", bufs=os_bufs))
        # bf16 spill buffers for output-projection partials computed inside
        # the last head-pair's (ACT-bound) attention window
        pool_sp = ctx.enter_context(tc.tile_pool(name="sp", bufs=4))
        ps_acc = ctx.enter_context(tc.tile_pool(name="psacc", bufs=acc_bufs, space="PSUM"))
        ps_att = ctx.enter_context(tc.tile_pool(name="psatt", bufs=att_bufs, space="PSUM"))

        # ---- constants / inputs ----
        # wq's t=0 slice lands first so the first projection matmul can
        # start as soon as x tile 0 arrives
        wq_sb = pool_w.tile([PT, KT, T, PT], bf16, tag="w", name="wq_sb")
        wk_sb = pool_w.tile([PT, KT, T, PT], bf16, tag="w", name="wk_sb")
        wv_sb = pool_w.tile([PT, KT, T, PT], bf16, tag="w", name="wv_sb")
        id_sb = singles.tile([PT, PT], bf16)
        mask_sb = singles.tile([PT, PT], f32)
        x_sb = [pool_xa.tile([PT, Sc], bf16, tag="xa", name=f"x{kt}")
                for kt in range(KT)]
        # HWDGE drains this queue in order: land the t=0 weight slices and
        # the first two x tiles before the bulk of x, so the q projection
        # starts immediately and the k/v groups never stall on their weights
        # first-MM latency: wq0 + half of x0 suffice for the first q matmul;
        # k/v weights follow behind the early x tiles (their matmuls trail
        # q's in the kt-interleaved order anyway)
        h1, h2 = Sc // 2, Sc
        nc.sync.dma_start(out=wq_sb[:, :, 0, :], in_=wq_d.ap()[:, :, 0, :])
        nc.sync.dma_start(out=x_sb[0][:, 0:h1], in_=xT_d.ap()[0][:, 0:h1])
        nc.sync.dma_start(out=wk_sb[:, :, 0, :], in_=wk_d.ap()[:, :, 0, :])
        nc.sync.dma_start(out=x_sb[0][:, h1:h2], in_=xT_d.ap()[0][:, h1:h2])
        nc.sync.dma_start(out=wv_sb[:, :, 0, :], in_=wv_d.ap()[:, :, 0, :])
        nc.sync.dma_start(out=x_sb[1][:, :], in_=xT_d.ap()[1])
        nc.sync.dma_start(out=id_sb[:, :], in_=id_d.ap())
        nc.sync.dma_start(out=mask_sb[:, :], in_=mask_d.ap())
        for kt in range(2, KT):
            nc.sync.dma_start(out=x_sb[kt][:, :], in_=xT_d.ap()[kt])
        nc.sync.dma_start(out=wq_sb[:, :, 1:T, :], in_=wq_d.ap()[:, :, 1:T, :])
        nc.sync.dma_start(out=wk_sb[:, :, 1:T, :], in_=wk_d.ap()[:, :, 1:T, :])
        nc.sync.dma_start(out=wv_sb[:, :, 1:T, :], in_=wv_d.ap()[:, :, 1:T, :])

        qT, kT_sb, v_sb, aT = [], [], [], []
        for t in range(T):
            qt = singles.tile([PT, Sc], bf16, name=f"qT{t}")
            kt_ = singles.tile([PT, Sc], bf16, name=f"kT{t}")
            vt_ = singles.tile([PT, NTT, 2 * PT], bf16, name=f"v{t}")
            qT.append(qt)
            kT_sb.append(kt_)
            v_sb.append(vt_)
            # ones columns for the [v|1] denominator trick (two bf16 ones
            # per u32). Written once; v copies only touch cols 0:64/128:192.
            nc.gpsimd.memset(
                vt_[:, :, :].rearrange("p g (h x) -> p g h x", x=PT)
                [:, :, :, DK:PT].bitcast(u32), 0x3F803F80)

        # HAM warmup: the PE clock gate starts at K=4/8 (1.2 GHz) and only
        # releases after ~3.4us of sustained activity; without help the
        # whole x-DMA-bound head phase (~17us) runs at half clock.  Dummy
        # matmuls (zeros stationary, never read, into the then-idle pso
        # PSUM slot) fill the DMA-wait gaps; the LDW-dedup pass strips
        # their redundant weight loads.
        dz = singles.tile([PT, PT], bf16, name="dz")
        nc.gpsimd.memset(dz[:, :].bitcast(u32), 0)
        dps = ps_att.tile([PT, CH], f32, tag="pso", name="dummy_ps")

        def warmup(n):
            for _ in range(n):
                nc.tensor.matmul(dps[:, 0:PT], dz[:, :], dz[:, :],
                                 start=True, stop=True,
                                 skip_group_check=True)

        # ---- projections (weight-stationary kt-outer) ----
        # Head-pair 0 runs up front with q/k/v kt-interleaved (3 concurrent
        # PSUM accumulators on the 2 "acc" + 1 "proj" slots) so the PE
        # tracks the incoming x DMA stream: one x tile feeds 6 matmuls
        # (~1.3us) before the next tile lands (~1.4us).  Head-pairs 1..3
        # are deferred into the attention pipeline: each (which, cpair)
        # group is a generator emitted one kt-slice per attention step on
        # the dedicated 1-slot "proj" PSUM tag, filling the PE idle slots
        # of the ACT-bound attention phase without stealing the score
        # tiles' slots or bursting ahead of the exp stream.
        def evac_qk(t, which, chunks, banks, dve):
            dst = qT[t] if which == "q" else kT_sb[t]
            for c in chunks:
                eng = (nc.vector.tensor_copy if (dve or c % 2)
                       else nc.scalar.copy)
                eng(dst[:, c * CH:(c + 1) * CH], banks[c])

        def evac_v_bank(c, bank):
            # v: evacuate to SBUF bf16 (frees the accumulator before the
            # PE-transposes so the transpose PSUM tile can reuse its slot)
            vt = pool_vt.tile([PT, CH], bf16, tag="vt")
            nc.vector.tensor_copy(vt[:, :], bank)
            return vt

        def evac_v_pst(t, c, vt, tag):
            # PE-transpose to natural [tok, ch] order, interleave into
            # [v_h0|1|v_h1|1]
            pst = ps_acc.tile([PT, 4, PT], bf16, tag=tag, name=f"tp{t}{c}",
                              bufs=(1 if tag == "proj" else None))
            for j in range(CH // PT):
                nc.tensor.transpose(
                    pst[:, j, :], vt[:, j * PT:(j + 1) * PT], id_sb[:, :])
            src = pst[:, :, :].rearrange("p j (h x) -> p j h x", x=DK)
            dst = v_sb[t][:, 4 * c:4 * c + 4, :].rearrange(
                "p j (h x) -> p j h x", x=PT)[:, :, :, 0:DK]
            nc.vector.tensor_copy(dst, src)

        def emit_upfront(t, cpair, warm=False):
            chunks = [c for c in cpair if c < NCH]
            bts = {}
            for which in ("q", "k", "v"):
                tag = "proj" if which == "v" else "acc"
                bts[which] = ps_acc.tile(
                    [PT, 2, CH], f32, tag=tag, name=f"pj{which}{t}{cpair[0]}",
                    bufs=(1 if tag == "proj" else None))
            for kt in range(KT):
                if warm:
                    # fill the x-DMA wait between kt slices, keeping the
                    # HAM activity window saturated (dummies cost ~107ns
                    # each; the x arrival jitter they absorb is ~1us+)
                    warmup(3)
                for which, wsb in (("q", wq_sb), ("k", wk_sb), ("v", wv_sb)):
                    for g, c in enumerate(chunks):
                        nc.tensor.matmul(
                            bts[which][:, g, :], wsb[:, kt, t, :],
                            x_sb[kt][:, c * CH:(c + 1) * CH],
                            start=(kt == 0), stop=(kt == KT - 1))
            for which in ("q", "k"):
                banks = {c: bts[which][:, g, :] for g, c in enumerate(chunks)}
                evac_qk(t, which, chunks, banks, False)
            vts = [evac_v_bank(c, bts["v"][:, g, :])
                   for g, c in enumerate(chunks)]
            for c, vt in zip(chunks, vts):
                evac_v_pst(t, c, vt, "proj")

        def proj_gen(t, which, cpair):
            """Deferred projection group: one kt-slice (or evac slice) per
            yield, driven by the attention pipeline's step loop."""
            wsb = {"q": wq_sb, "k": wk_sb, "v": wv_sb}[which]
            chunks = [c for c in cpair if c < NCH]
            bt = ps_acc.tile([PT, 2, CH], f32, tag="proj",
                             name=f"pj{which}{t}{cpair[0]}", bufs=1)
            banks = {c: bt[:, g, :] for g, c in enumerate(chunks)}
            for kt in range(KT):
                for c in chunks:
                    nc.tensor.matmul(
                        banks[c], wsb[:, kt, t, :],
                        x_sb[kt][:, c * CH:(c + 1) * CH],
                        start=(kt == 0), stop=(kt == KT - 1))
                yield
            if which in ("q", "k"):
                evac_qk(t, which, chunks, banks, True)
                yield
            else:
                vts = [evac_v_bank(c, banks[c]) for c in chunks]
                yield
                for c, vt in zip(chunks, vts):
                    evac_v_pst(t, c, vt, "proj")
                    yield

        def n_slices(which, cpair):
            nch = len([c for c in cpair if c < NCH])
            return KT + (1 if which in ("q", "k") else 1 + nch)

        cpairs = [(0, 1), (2, 3)] if NCH > 1 else [(0,)]
        projq = []
        # ~4.3us of back-to-back dummies trips the HAM clock gate to
        # K=8/8 before the first real projection matmuls (which would
        # otherwise run the whole DMA-paced head phase at 1.2 GHz)
        warmup(40)
        for t in range(T):
            if t == 0 or not do_attn:
                for cp in cpairs:
                    # t0's (2,3) chunks defer into t0's early attention
                    # steps (chunks 0/1 don't read them; the ramp-in steps
                    # are thin and would otherwise idle the PE)
                    if t == 0 and do_attn and NCH > 2 and cp[0] >= 2:
                        for which in ("q", "k", "v"):
                            projq.append((0, which, cp))
                    else:
                        emit_upfront(t, cp, warm=(t == 0 and cp[0] == 0))
            else:
                for which in ("q", "k", "v"):
                    for cp in cpairs:
                        projq.append((t, which, cp))

        for t in range(T):
            aT.append(singles.tile([PT, Sc], bf16, name=f"aT{t}"))
        wo_sb = pool_w.tile([PT, T, NT, PT], bf16, tag="w", name="wo_sb")
        nc.sync.dma_start(out=wo_sb[:, :, :, :], in_=wo_d.ap())

        # ---- phase 2: attention, one rolling software pipeline ----
        # A single S-stream (scores+mask+exp) runs LA steps ahead of the
        # A-stream (AV accumulation) across ALL (head-pair, chunk, key-tile)
        # steps, so the pipeline never drains at chunk or head-pair
        # boundaries and the PE never waits on the ACT engine's exp.
        LA = acc_bufs  # scores lookahead (steps) = pss slot count
        KC = CH // PT
        allsteps = [(t, c, kt)
                    for t in range(T if do_attn else 0)
                    for c in range(NCH)
                    for kt in range((c + 1) * KC)]
        pso_by_tc = {}
        nmw_by_t, dnw_by_t, lg_by_t, dm_by_t = {}, {}, {}, {}

        def emit_S(idx):
            t, c, kt = allsteps[idx]
            i = kt - c * KC            # >=0 on the diagonal superblock
            off = max(i, 0) * PT       # queries < off are fully masked
            if kt == 0:
                pso_by_tc[(t, c)] = {
                    h: ps_att.tile([PT, CH], f32, tag="pso", name=f"pso{t}{c}{h}")
                    for h in (0, 1)}
            pss = ps_acc.tile([PT, 2, CH], f32, tag="acc", name=f"ss{t}{c}{kt}")
            for h in (0, 1):
                hp = slice(h * DK, (h + 1) * DK)
                nc.tensor.matmul(
                    pss[:, h, off:CH],
                    kT_sb[t][hp, kt * PT:(kt + 1) * PT],
                    qT[t][hp, c * CH + off:(c + 1) * CH],
                    start=True, stop=True)
            if i >= 0:
                for h in (0, 1):
                    nc.vector.tensor_add(
                        pss[:, h, off:off + PT], pss[:, h, off:off + PT],
                        mask_sb[:, :])
            Pt = pool_P.tile([PT, 2, CH], bf16, tag="P", name=f"P{t}{c}{kt}")
            nc.scalar.activation(
                out=Pt[:, :, off:CH], in_=pss[:, :, off:CH], func=AF.Exp)
            return Pt, off

        def emit_A(idx, Pt, off):
            t, c, kt = allsteps[idx]
            nkt = (c + 1) * KC
            pso = pso_by_tc[(t, c)]
            for h in (0, 1):
                nc.tensor.matmul(
                    pso[h][:, off:CH],
                    v_sb[t][:, kt, h * PT:(h + 1) * PT],
                    Pt[:, h, off:CH],
                    start=(kt == 0), stop=(kt == nkt - 1),
                    skip_group_check=True)
            if kt == nkt - 1:
                # chunk done: evacuate numerators + denominators into the
                # per-head-pair wide tiles (copy-first frees the pso banks
                # immediately); the Ln/Exp/mul happen once per head-pair.
                if t not in nmw_by_t:
                    nmw_by_t[t] = pool_nmw.tile([PT, NCH, CH], f32, tag="nmw",
                                                name=f"nmw{t}")
                    dnw_by_t[t] = pool_dn.tile([PT, NCH, CH], f32, tag="dn",
                                               name=f"dnw{t}")
                nmw, dnw = nmw_by_t[t], dnw_by_t[t]
                for h in (0, 1):
                    hr = slice(h * DK, (h + 1) * DK)
                    nc.vector.tensor_copy(nmw[hr, c, :], pso[h][0:DK, :])
                    nc.vector.tensor_copy(dnw[hr, c, :], pso[h][DK:2 * DK, :])
                del pso_by_tc[(t, c)]
                last_of_t = (idx + 1 == len(allsteps)
                             or allsteps[idx + 1][0] != t)
                if t == T - 1:
                    # last head-pair: normalize per chunk so only the final
                    # chunk's small Ln/Exp sits between the last AV and the
                    # output projection's first ct=3 matmul
                    if t not in lg_by_t:
                        lg_by_t[t] = pool_dn.tile([PT, NCH, CH], f32,
                                                  tag="dn", name=f"lgw{t}")
                        dm_by_t[t] = pool_dmw.tile([PT, NCH, CH], bf16,
                                                   tag="dmw", name=f"dmw{t}")
                    lgw, dmw = lg_by_t[t], dm_by_t[t]
                    nc.scalar.activation(out=lgw[:, c, :], in_=dnw[:, c, :],
                                         func=AF.Ln)
                    nc.scalar.activation(out=dmw[:, c, :], in_=lgw[:, c, :],
                                         func=AF.Exp, scale=-1.0)
                    nc.vector.tensor_mul(
                        aT[t][:, c * CH:(c + 1) * CH],
                        nmw[:, c, :], dmw[:, c, :])
                    if last_of_t:
                        del nmw_by_t[t], dnw_by_t[t]
                    return
                if last_of_t:
                    # one batched Ln/Exp/mul per head-pair (the ACT engine
                    # is the attention bottleneck: 2 calls of N=2048 replace
                    # 8 calls of N=512)
                    lgw = pool_dn.tile([PT, NCH, CH], f32, tag="dn",
                                       name=f"lgw{t}")
                    # dmw in bf16: 1/denominator at 0.4% rel err is well
                    # within the gate and halves this pool's SBUF footprint
                    dmw = pool_dmw.tile([PT, NCH, CH], bf16, tag="dmw",
                                        name=f"dmw{t}")
                    nc.scalar.activation(out=lgw[:, :, :], in_=dnw[:, :, :],
                                         func=AF.Ln)
                    nc.scalar.activation(out=dmw[:, :, :], in_=lgw[:, :, :],
                                         func=AF.Exp, scale=-1.0)
                    nc.vector.tensor_mul(
                        aT[t][:, 0:NCH * CH].rearrange(
                            "p (c q) -> p c q", c=NCH),
                        nmw[:, :, :], dmw[:, :, :])
                if last_of_t:
                    del nmw_by_t[t], dnw_by_t[t]

        # pace the deferred projection sub-groups into the pipeline:
        # head-pair t's sub-groups are spread across the attention steps of
        # head-pair t-1, so each head-pair's projections finish (with
        # lookahead margin) before the attention steps that consume them
        first_step = {}
        for idx, (t_, _, _) in enumerate(allsteps):
            first_step.setdefault(t_, idx)
        gens_at = {}
        for tq in range(0, T):
            if not allsteps:
                break
            if tq == 0:
                # t0's deferred (2,3) groups must finish before the first
                # step that reads chunk 2 (start of chunk c=2), with margin
                lo = 0
                hi = max(allsteps.index((0, 2, 0)) - LA - 2, 0) \
                    if (0, 2, 0) in allsteps else 0
            else:
                lo = first_step[tq - 1]
                hi = max(first_step[tq] - LA - 2, lo)
            grp = [g for g in projq if g[0] == tq]
            if not grp:
                continue
            span = max(hi - lo, 1) / len(grp)
            for j, g in enumerate(grp):
                gen = proj_gen(*g)
                nsl = n_slices(g[1], g[2])
                for sl in range(nsl):
                    step = min(int(lo + span * j + span * (sl + 1) / nsl), hi)
                    gens_at.setdefault(step, []).append(gen)
        # during the last head-pair's attention there are no deferred
        # projections left and the window is ACT(exp)-bound: fill the free
        # "proj" PSUM slot with the first few output-projection groups'
        # ct=0..2 partials, spilled to SBUF as bf16 (phase 3 adds them back
        # during evacuation).
        spilled = {}

        def op_spill_gen(nt, cp):
            chunks = [c for c in cp if c < NCH]
            bt = ps_acc.tile([PT, 2, CH], f32, tag="proj",
                             name=f"osp{nt}{cp[0]}", bufs=1)
            banks = {c: bt[:, g, :] for g, c in enumerate(chunks)}
            for ct in range(T - 1):
                for c in chunks:
                    nc.tensor.matmul(
                        banks[c], wo_sb[:, ct, nt, :],
                        aT[ct][:, c * CH:(c + 1) * CH],
                        start=(ct == 0), stop=(ct == T - 2),
                        skip_group_check=True)
                yield
            sp = pool_sp.tile([PT, 2, CH], bf16, tag="sp",
                              name=f"sp{nt}{cp[0]}")
            nc.vector.tensor_copy(sp[:, 0:len(chunks), :],
                                  bt[:, 0:len(chunks), :])
            spilled[(nt, cp)] = sp
            yield

        ogroups = ([(nt, cp) for nt in range(NT) for cp in cpairs]
                   if (do_attn and do_outproj) else [])
        spill_groups = ogroups[:4] if (do_attn and do_outproj and T > 1) else []
        spill_gens = []
        if spill_groups and allsteps:
            lo3 = first_step[T - 1] + 2
            hi3 = max(len(allsteps) - 2, lo3)
            span3 = max(hi3 - lo3, 1) / len(spill_groups)
            for j, g in enumerate(spill_groups):
                gen = op_spill_gen(*g)
                spill_gens.append(gen)
                for sl in range(T):
                    step = min(int(lo3 + span3 * j + span3 * (sl + 1) / T), hi3)
                    gens_at.setdefault(step, []).append(gen)

        inflight = {}
        for j in range(min(LA, len(allsteps))):
            inflight[j] = emit_S(j)
        for i in range(len(allsteps)):
            for gen in gens_at.get(i, []):
                next(gen, None)
            if i + LA < len(allsteps):
                inflight[i + LA] = emit_S(i + LA)
            emit_A(i, *inflight.pop(i))
        for gen in spill_gens:
            for _ in gen:
                pass

        # ---- phase 3: output projection (pipelined, partial, transposed) ----
        # (nt, chunk-pair) groups run a 2-deep software pipeline on the two
        # "acc" slots: group g's ct=0..2 accumulation runs while group g-1
        # finishes ct=3 (which waits on the last head-pair's normalization)
        # and evacuates, hiding that tail and spreading the output DMA.
        def op_start(nt, cp, tag):
            chunks = [c for c in cp if c < NCH]
            bt = ps_acc.tile([PT, 2, CH], f32, tag=tag, name=f"op{nt}{cp[0]}",
                             bufs=(1 if tag == "proj" else None))
            banks = {c: bt[:, g, :] for g, c in enumerate(chunks)}
            for ct in range(T - 1):
                for c in chunks:
                    nc.tensor.matmul(
                        banks[c], wo_sb[:, ct, nt, :],
                        aT[ct][:, c * CH:(c + 1) * CH],
                        start=(ct == 0), stop=False, skip_group_check=True)
            return banks

        def op_finish(nt, cp, banks, sp=None):
            chunks = [c for c in cp if c < NCH]
            for c in chunks:
                nc.tensor.matmul(
                    banks[c], wo_sb[:, T - 1, nt, :],
                    aT[T - 1][:, c * CH:(c + 1) * CH],
                    start=(sp is not None), stop=True, skip_group_check=True)
            ost = pool_os.tile([PT, 2, CH], bf16, tag="os")
            for g, c in enumerate(chunks):
                if sp is not None:
                    nc.vector.tensor_add(ost[:, g, :], banks[c], sp[:, g, :])
                else:
                    eng = (nc.scalar.copy if c % 2 == 0
                           else nc.vector.tensor_copy)
                    eng(ost[:, g, :], banks[c])
            nc.sync.dma_start(
                out=out_d.ap()[nt * PT:(nt + 1) * PT,
                               chunks[0] * CH:(chunks[-1] + 1) * CH],
                in_=ost[:, 0:len(chunks), :])

        # 3-deep pipeline: the "proj" slot is free once the attention-window
        # spills evacuated, so op-group tiles cycle acc/acc/proj.  Spilled
        # groups only need their ct=3 matmul (fresh accumulation) plus an
        # add-back of the bf16 partial during evacuation.
        pend = []
        for j, (nt, cp) in enumerate(ogroups):
            sp = spilled.get((nt, cp))
            if sp is not None:
                bt = ps_acc.tile([PT, 2, CH], f32, tag="acc",
                                 name=f"opf{nt}{cp[0]}")
                chunks = [c for c in cp if c < NCH]
                banks = {c: bt[:, g, :] for g, c in enumerate(chunks)}
                pend.append((nt, cp, banks, sp))
            else:
                banks = op_start(nt, cp, "proj" if j % 3 == 2 else "acc")
                pend.append((nt, cp, banks, None))
            if len(pend) > 2:
                op_finish(*pend.pop(0))
        for p in pend:
            op_finish(*p)

    _dedup_ldweights(nc)
    if split_waits:
        _split_multi_waits(nc)
    return nc


_build_cache = {}


def _get_program(Sc=S):
    key = Sc
    if key not in _build_cache:
        _build_cache[key] = build(Sc)
    return _build_cache[key]


def _bf16(a):
    import ml_dtypes
    return np.ascontiguousarray(a).astype(ml_dtypes.bfloat16)


def make_in_maps(x, w_q, w_k, w_v, w_o):
    """Host-side sharding: returns per-core input dicts.
    Core c: batch c//2, head-half c%2."""
    Bc, Sc, Dc = x.shape
    scale = DK ** -0.5
    ident = np.eye(PT, dtype=np.float32)
    jj, qq = np.meshgrid(np.arange(PT), np.arange(PT), indexing="ij")
    mask = np.where(jj <= qq, 0.0, NEG).astype(np.float32)

    def pack_w(w):  # [1024, 512] -> [128 p, 8 kt, 4 t, 128 c]
        return np.ascontiguousarray(
            w.reshape(KT, PT, T, PT).transpose(1, 0, 2, 3))

    xTs = [_bf16(x[b].T.reshape(KT, PT, Sc)) for b in range(Bc)]
    whalf = []
    for hh in range(2):
        rows = slice(512 * hh, 512 * hh + 512)
        wo_half = w_o[:, rows].T.reshape(T, PT, NT, PT).transpose(1, 0, 2, 3)
        whalf.append({
            "wq": _bf16(pack_w((w_q[rows, :] * scale).T)),
            "wk": _bf16(pack_w(w_k[rows, :].T)),
            "wv": _bf16(pack_w(w_v[rows, :].T)),
            "wo": _bf16(np.ascontiguousarray(wo_half)),
        })
    in_maps = []
    for c in range(NCORES):
        b, hh = c // 2, c % 2
        m = {"xT": xTs[b], "ident": _bf16(ident), "mask": mask}
        m.update(whalf[hh])
        in_maps.append(m)
    return in_maps


def run_on_hw(in_maps, Sc=S, trace=False, trace_cores=None):
    nc = _get_program(Sc)
    return bass_utils.run_bass_kernel_spmd(
        nc, in_maps, core_ids=list(range(NCORES)), trace=trace,
        trace_cores=trace_cores)


def kernel(x, w_q, w_k, w_v, w_o, b_o):
    x = np.asarray(x, dtype=np.float32)
    w_q = np.asarray(w_q, dtype=np.float32)
    w_k = np.asarray(w_k, dtype=np.float32)
    w_v = np.asarray(w_v, dtype=np.float32)
    w_o = np.asarray(w_o, dtype=np.float32)
    b_o = np.asarray(b_o, dtype=np.float32)
    Bc, Sc, Dc = x.shape
    in_maps = make_in_maps(x, w_q, w_k, w_v, w_o)
    res = run_on_hw(in_maps, Sc)
    out = np.empty((Bc, Sc, Dc), dtype=np.float32)
    for b in range(Bc):
        outT = (np.asarray(res.results[2 * b]["outT"], dtype=np.float32)
                + np.asarray(res.results[2 * b + 1]["outT"], dtype=np.float32))
        out[b] = outT.T + b_o
    return out



# revision 44
# speedup vs baseline: 1.0125x; 1.0125x over previous
"""Causal multi-head attention for Trainium2, 8-core (batch x head-half) parallel.

Problem: B=4, S=2048, D=1024, H=16 heads (dk=64), fp32 in/out.
    q = x @ w_q.T ; k = x @ w_k.T ; v = x @ w_v.T   (per-head split)
    out = softmax(causal(q k^T / 8)) v, concat heads, @ w_o.T + b_o

Sharding: core c owns batch b = c//2 and head-half hh = c%2 (8 heads =
channels [512*hh, 512*hh+512)).  Each core computes q/k/v projections for
its 512 channels over its one batch, runs causal attention for its 8 heads,
and produces a partial output projection outT_c = w_o[:, ch]^T a_c^T of
shape [1024, S]; the host sums core pairs (2b, 2b+1), transposes, adds b_o.

All matmul operands are bf16 (fp32 PSUM accumulation; validated 3.8e-3 max
rel err in numpy vs the 2e-2 gate).  bf16 keeps the PE at 1 cycle/row,
enables FWL fast weight loads, and halves SBUF/DMA vs fp32.

Per-core dataflow (head-pair t = 0..3 maps to SBUF partition tiles):
  - x is pre-transposed + bf16 on host: xT [8, 128, S] so the contraction
    dim D lands on SBUF partitions.
  - projections run weight-stationary kt-outer: one LDWEIGHTS feeds 4
    matmuls (one per 512-token chunk), accumulating in 4 PSUM banks.
  - v is PE-transposed to natural [tok, ch] order and stored as
    [v_h | ones] stationaries: AV then yields both the attention output
    (rows 0-63) and the softmax denominator replicated on rows 64-127.
  - scores are computed transposed (keys on partitions): sT = kT^T qT with
    two heads running concurrently in PE row groups 0-1 / 2-3.
  - softmax without max-subtraction (scores ~N(0,1); exp in fp32 PSUM),
    causal handled by an additive -1e9 triangle mask on exact-diagonal
    128x128 blocks; above-diagonal work inside a diagonal 512-superblock is
    skipped by trimming the matmul free dim (queries < 128*i are never
    computed or exp'd, and the AV accumulation never reads them).
  - normalization: numerators/denominators are evacuated copy-first into
    per-head-pair wide tiles (frees PSUM fast); ONE batched Ln + Exp
    (1/x = exp(-ln x), both functions in one ACT table set) + one DVE mul
    per head-pair replaces 8 small ACT calls (ACT is the attention-phase
    bottleneck).  The last head-pair normalizes per chunk instead so only
    the final chunk's small Ln/Exp separates attention from the output
    projection.
  - attention is a single rolling software pipeline over all (head-pair,
    chunk, key-tile) steps with LA=2 score-tiles of lookahead.  Deferred
    work fills the PE whenever the window is ACT-bound: head-pair t's
    q/k/v projections run one kt-slice per step of head-pair t-1's
    attention on a dedicated 1-slot "proj" PSUM tag (t0's chunks 2-3
    fill t0's own thin ramp-in); the first 4 output-projection groups'
    ct=0..2 partials run inside the last head-pair's window and spill to
    SBUF bf16.  PSUM: 2x2-bank score slots + 1x2-bank proj slot +
    2x1-bank AV accumulators = exactly 8 banks.
  - a post-pass NoOp-ifies LDWEIGHTS instructions identical to their
    predecessor (bass re-emits one per matmul; each costs ~95ns of dead
    PE time since FWL/background weight buffering is absent), and the
    output projection runs a 3-deep software pipeline of (row-tile,
    chunk-pair) groups with bf16 outputs DMA'd per group (host sums the
    two per-batch partials in fp32).
  - measured (neuron-profile over 8 cores): ~321.6 us max / ~316 us mean
    vs 334.7 us for the previous all-upfront schedule (7.06 ms for the
    original fp32 head-sharded baseline); rel err 4.4e-3 vs the 2e-2
    gate.  fp8 DoubleRow was evaluated and rejected: e4m3 operand
    quantization exceeds the accuracy gate (8e-2 on projections alone).
"""

import numpy as np

import concourse.bass as bass
import concourse.tile as tile
from concourse import mybir
from concourse import bass_utils

f32 = mybir.dt.float32
f32r = mybir.dt.float32r
bf16 = mybir.dt.bfloat16
fp16 = mybir.dt.float16
u32 = mybir.dt.uint32
AF = mybir.ActivationFunctionType

B, S, D, H = 4, 2048, 1024, 16
DK = D // H            # 64
NCORES = 8
PT = 128               # partition tile
CH = 512               # query chunk (PSUM bank = 512 fp32)
KT = D // PT           # 8 contraction tiles over D
T = 4                  # head-pairs per core (8 heads)
NT = D // PT           # 8 output row tiles for the o-projection
NEG = -1.0e9


def _split_multi_waits(nc):
    """This walrus build allows at most one sync-wait per TPB instruction;
    hoist extra waits onto single-wait NoOps on the same engine."""
    n = 0
    for f in nc.m.functions:
        for blk in f.blocks:
            new = []
            for inst in blk.instructions:
                si = inst.sync_info
                if si is not None and si.on_wait and len(si.on_wait) > 1:
                    ws = list(si.on_wait)
                    for w in ws[:-1]:
                        new.append(mybir.InstNoOp(
                            name=f"I-wfix-{n}", ins=[], outs=[], engine=inst.engine,
                            sync_info=mybir.SyncInfo(on_wait=[w], on_update=[])))
                        n += 1
                    inst.sync_info = mybir.SyncInfo(
                        on_wait=[ws[-1]], on_update=list(si.on_update))
                new.append(inst)
            blk.instructions = new
    return n


def _dedup_ldweights(nc):
    """The PE pays ~95ns of dead time per LDWEIGHTS (no FWL / background
    buffer in this build), and bass re-emits an identical LDWEIGHTS for
    every matmul even when consecutive matmuls share a stationary (e.g.
    the two chunk-matmuls of one projection kt-slice).  Replace a
    Tensor-queue LDWEIGHTS whose operand is byte-identical to the
    immediately-preceding one (with only matmuls/noops/semaphores in
    between, and those matmuls not in transpose mode) by a NoOp carrying
    the same sync_info.  All stationary tiles here are write-once, so an
    identical AP means identical weights."""
    n = 0
    for f in nc.m.functions:
        for blk in f.blocks:
            last_sig = None
            pe_eng = None
            new = []
            for inst in blk.instructions:
                if isinstance(inst, mybir.InstLdweights):
                    pe_eng = inst.engine
                    sig = (str(inst.ins[0]),
                           bool(inst.is_transpose), str(inst.perf_mode),
                           str(inst.tile_position))
                    if sig == last_sig:
                        si = inst.sync_info
                        if si is not None and (si.on_wait or si.on_update):
                            new.append(mybir.InstNoOp(
                                name=f"I-lwdup-{n}", ins=[], outs=[],
                                engine=inst.engine, sync_info=si))
                        n += 1
                        continue
                    last_sig = sig
                elif inst.engine != pe_eng:
                    # other engines don't touch the PE weight registers (all
                    # stationary tiles in this kernel are write-once before
                    # first use, so cross-engine writes can't invalidate)
                    pass
                elif isinstance(inst, (mybir.InstMatmult, mybir.InstNoOp,
                                       mybir.InstEventSemaphore)):
                    if getattr(inst, "is_transpose", False):
                        last_sig = None
                else:
                    last_sig = None
                new.append(inst)
            blk.instructions = new
    return n


def build(Sc=S, split_waits=True, p_bufs=6, vt_bufs=2, dm_bufs=2, nm_bufs=3,
          os_bufs=4, acc_bufs=2, att_bufs=2, do_attn=True, do_outproj=True):
    """Build the per-core Bass program. Same program for all 8 cores; only
    the input data differs per core."""
    from contextlib import ExitStack

    NCH = Sc // CH         # query chunks
    NTT = Sc // PT         # token/key tiles

    nc = bass.Bass("TRN2", target_bir_lowering=False, debug=False)

    xT_d = nc.dram_tensor("xT", [KT, PT, Sc], bf16, kind="ExternalInput")
    wq_d = nc.dram_tensor("wq", [PT, KT, T, PT], bf16, kind="ExternalInput")
    wk_d = nc.dram_tensor("wk", [PT, KT, T, PT], bf16, kind="ExternalInput")
    wv_d = nc.dram_tensor("wv", [PT, KT, T, PT], bf16, kind="ExternalInput")
    wo_d = nc.dram_tensor("wo", [PT, T, NT, PT], bf16, kind="ExternalInput")
    id_d = nc.dram_tensor("ident", [PT, PT], bf16, kind="ExternalInput")
    mask_d = nc.dram_tensor("mask", [PT, PT], f32, kind="ExternalInput")
    # bf16 partial outputs: halves the 8MB/core output DMA (host sums the
    # two per-batch partials in fp32; bf16 rounding adds ~4e-4 to the gate)
    out_d = nc.dram_tensor("outT", [D, Sc], bf16, kind="ExternalOutput")

    with tile.TileContext(nc) as tc, ExitStack() as ctx:
        singles = ctx.enter_context(tc.tile_pool(name="singles", bufs=1))
        # phase-limited tensors share slots: wq/wk/wv (phase 1) and wo
        # (phase 3) rotate through 3 slots; the 8 x tiles (phase 1) and the
        # 4 aT tiles (phases 2-3) rotate through 8 slots.
        pool_w = ctx.enter_context(tc.tile_pool(name="w", bufs=3))
        pool_xa = ctx.enter_context(tc.tile_pool(name="xa", bufs=8))
        pool_P = ctx.enter_context(tc.tile_pool(name="P", bufs=p_bufs))
        pool_vt = ctx.enter_context(tc.tile_pool(name="vt", bufs=vt_bufs))
        # per-head-pair wide normalization tiles: denominators and numerators
        # for all NCH chunks are collected, then ONE Ln + ONE Exp + ONE mul
        # per head-pair replaces the 4-per-chunk ACT calls (the ACT engine is
        # the attention-phase bottleneck; this removes ~30us of ACT work and
        # keeps the small norm calls out of the critical exp stream).
        pool_dn = ctx.enter_context(tc.tile_pool(name="dn", bufs=2))
        pool_nmw = ctx.enter_context(tc.tile_pool(name="nmw", bufs=2))
        pool_dmw = ctx.enter_context(tc.tile_pool(name="dmw", bufs=2))
        pool_os = ctx.enter_context(tc.tile_pool(name="os", bufs=os_bufs))
        # bf16 spill buffers for output-projection partials computed inside
        # the last head-pair's (ACT-bound) attention window
        pool_sp = ctx.enter_context(tc.tile_pool(name="sp", bufs=4))
        ps_acc = ctx.enter_context(tc.tile_pool(name="psacc", bufs=acc_bufs, space="PSUM"))
        ps_att = ctx.enter_context(tc.tile_pool(name="psatt", bufs=att_bufs, space="PSUM"))

        # ---- constants / inputs ----
        # wq's t=0 slice lands first so the first projection matmul can
        # start as soon as x tile 0 arrives
        wq_sb = pool_w.tile([PT, KT, T, PT], bf16, tag="w", name="wq_sb")
        wk_sb = pool_w.tile([PT, KT, T, PT], bf16, tag="w", name="wk_sb")
        wv_sb = pool_w.tile([PT, KT, T, PT], bf16, tag="w", name="wv_sb")
        id_sb = singles.tile([PT, PT], bf16)
        mask_sb = singles.tile([PT, PT], f32)
        x_sb = [pool_xa.tile([PT, Sc], bf16, tag="xa", name=f"x{kt}")
                for kt in range(KT)]
        # HWDGE drains this queue in order: land the t=0 weight slices and
        # the first two x tiles before the bulk of x, so the q projection
        # starts immediately and the k/v groups never stall on their weights
        # first-MM latency: wq0 + half of x0 suffice for the first q matmul;
        # k/v weights follow behind the early x tiles (their matmuls trail
        # q's in the kt-interleaved order anyway)
        h1, h2 = Sc // 2, Sc
        nc.sync.dma_start(out=wq_sb[:, :, 0, :], in_=wq_d.ap()[:, :, 0, :])
        nc.sync.dma_start(out=x_sb[0][:, 0:h1], in_=xT_d.ap()[0][:, 0:h1])
        nc.sync.dma_start(out=wk_sb[:, :, 0, :], in_=wk_d.ap()[:, :, 0, :])
        nc.sync.dma_start(out=x_sb[0][:, h1:h2], in_=xT_d.ap()[0][:, h1:h2])
        nc.sync.dma_start(out=wv_sb[:, :, 0, :], in_=wv_d.ap()[:, :, 0, :])
        nc.sync.dma_start(out=x_sb[1][:, :], in_=xT_d.ap()[1])
        nc.sync.dma_start(out=id_sb[:, :], in_=id_d.ap())
        nc.sync.dma_start(out=mask_sb[:, :], in_=mask_d.ap())
        for kt in range(2, KT):
            nc.sync.dma_start(out=x_sb[kt][:, :], in_=xT_d.ap()[kt])
        nc.sync.dma_start(out=wq_sb[:, :, 1:T, :], in_=wq_d.ap()[:, :, 1:T, :])
        nc.sync.dma_start(out=wk_sb[:, :, 1:T, :], in_=wk_d.ap()[:, :, 1:T, :])
        nc.sync.dma_start(out=wv_sb[:, :, 1:T, :], in_=wv_d.ap()[:, :, 1:T, :])

        qT, kT_sb, v_sb, aT = [], [], [], []
        for t in range(T):
            qt = singles.tile([PT, Sc], bf16, name=f"qT{t}")
            kt_ = singles.tile([PT, Sc], bf16, name=f"kT{t}")
            vt_ = singles.tile([PT, NTT, 2 * PT], bf16, name=f"v{t}")
            qT.append(qt)
            kT_sb.append(kt_)
            v_sb.append(vt_)
            # ones columns for the [v|1] denominator trick (two bf16 ones
            # per u32). Written once; v copies only touch cols 0:64/128:192.
            nc.gpsimd.memset(
                vt_[:, :, :].rearrange("p g (h x) -> p g h x", x=PT)
                [:, :, :, DK:PT].bitcast(u32), 0x3F803F80)

        # HAM warmup: the PE clock gate starts at K=4/8 (1.2 GHz) and only
        # releases after ~3.4us of sustained activity; without help the
        # whole x-DMA-bound head phase (~17us) runs at half clock.  Dummy
        # matmuls (zeros stationary, never read, into the then-idle pso
        # PSUM slot) fill the DMA-wait gaps; the LDW-dedup pass strips
        # their redundant weight loads.
        dz = singles.tile([PT, PT], bf16, name="dz")
        nc.gpsimd.memset(dz[:, :].bitcast(u32), 0)
        dps = ps_att.tile([PT, CH], f32, tag="pso", name="dummy_ps")

        def warmup(n):
            for _ in range(n):
                nc.tensor.matmul(dps[:, 0:PT], dz[:, :], dz[:, :],
                                 start=True, stop=True,
                                 skip_group_check=True)

        # ---- projections (weight-stationary kt-outer) ----
        # Head-pair 0 runs up front with q/k/v kt-interleaved (3 concurrent
        # PSUM accumulators on the 2 "acc" + 1 "proj" slots) so the PE
        # tracks the incoming x DMA stream: one x tile feeds 6 matmuls
        # (~1.3us) before the next tile lands (~1.4us).  Head-pairs 1..3
        # are deferred into the attention pipeline: each (which, cpair)
        # group is a generator emitted one kt-slice per attention step on
        # the dedicated 1-slot "proj" PSUM tag, filling the PE idle slots
        # of the ACT-bound attention phase without stealing the score
        # tiles' slots or bursting ahead of the exp stream.
        def evac_qk(t, which, chunks, banks, dve):
            dst = qT[t] if which == "q" else kT_sb[t]
            for c in chunks:
                eng = (nc.vector.tensor_copy if (dve or c % 2)
                       else nc.scalar.copy)
                eng(dst[:, c * CH:(c + 1) * CH], banks[c])

        def evac_v_bank(c, bank):
            # v: evacuate to SBUF bf16 (frees the accumulator before the
            # PE-transposes so the transpose PSUM tile can reuse its slot)
            vt = pool_vt.tile([PT, CH], bf16, tag="vt")
            nc.vector.tensor_copy(vt[:, :], bank)
            return vt

        def evac_v_pst(t, c, vt, tag):
            # PE-transpose to natural [tok, ch] order, interleave into
            # [v_h0|1|v_h1|1]
            pst = ps_acc.tile([PT, 4, PT], bf16, tag=tag, name=f"tp{t}{c}",
                              bufs=(1 if tag == "proj" else None))
            for j in range(CH // PT):
                nc.tensor.transpose(
                    pst[:, j, :], vt[:, j * PT:(j + 1) * PT], id_sb[:, :])
            src = pst[:, :, :].rearrange("p j (h x) -> p j h x", x=DK)
            dst = v_sb[t][:, 4 * c:4 * c + 4, :].rearrange(
                "p j (h x) -> p j h x", x=PT)[:, :, :, 0:DK]
            nc.vector.tensor_copy(dst, src)

        def emit_upfront(t, cpair, warm=False):
            chunks = [c for c in cpair if c < NCH]
            bts = {}
            for which in ("q", "k", "v"):
                tag = "proj" if which == "v" else "acc"
                bts[which] = ps_acc.tile(
                    [PT, 2, CH], f32, tag=tag, name=f"pj{which}{t}{cpair[0]}",
                    bufs=(1 if tag == "proj" else None))
            for kt in range(KT):
                if warm and kt < 6:
                    # fill the x-DMA wait between kt slices, keeping the
                    # HAM activity window saturated
                    warmup(2)
                for which, wsb in (("q", wq_sb), ("k", wk_sb), ("v", wv_sb)):
                    for g, c in enumerate(chunks):
                        nc.tensor.matmul(
                            bts[which][:, g, :], wsb[:, kt, t, :],
                            x_sb[kt][:, c * CH:(c + 1) * CH],
                            start=(kt == 0), stop=(kt == KT - 1))
            for which in ("q", "k"):
                banks = {c: bts[which][:, g, :] for g, c in enumerate(chunks)}
                evac_qk(t, which, chunks, banks, False)
            vts = [evac_v_bank(c, bts["v"][:, g, :])
                   for g, c in enumerate(chunks)]
            for c, vt in zip(chunks, vts):
                evac_v_pst(t, c, vt, "proj")

        def proj_gen(t, which, cpair):
            """Deferred projection group: one kt-slice (or evac slice) per
            yield, driven by the attention pipeline's step loop."""
            wsb = {"q": wq_sb, "k": wk_sb, "v": wv_sb}[which]
            chunks = [c for c in cpair if c < NCH]
            bt = ps_acc.tile([PT, 2, CH], f32, tag="proj",
                             name=f"pj{which}{t}{cpair[0]}", bufs=1)
            banks = {c: bt[:, g, :] for g, c in enumerate(chunks)}
            for kt in range(KT):
                for c in chunks:
                    nc.tensor.matmul(
                        banks[c], wsb[:, kt, t, :],
                        x_sb[kt][:, c * CH:(c + 1) * CH],
                        start=(kt == 0), stop=(kt == KT - 1))
                yield
            if which in ("q", "k"):
                evac_qk(t, which, chunks, banks, True)
                yield
            else:
                vts = [evac_v_bank(c, banks[c]) for c in chunks]
                yield
                for c, vt in zip(chunks, vts):
                    evac_v_pst(t, c, vt, "proj")
                    yield

        def n_slices(which, cpair):
            nch = len([c for c in cpair if c < NCH])
            return KT + (1 if which in ("q", "k") else 1 + nch)

        cpairs = [(0, 1), (2, 3)] if NCH > 1 else [(0,)]
        projq = []
        # ~4.3us of back-to-back dummies trips the HAM clock gate to
        # K=8/8 before the first real projection matmuls (which would
        # otherwise run the whole DMA-paced head phase at 1.2 GHz)
        warmup(40)
        for t in range(T):
            if t == 0 or not do_attn:
                for cp in cpairs:
                    # t0's (2,3) chunks defer into t0's early attention
                    # steps (chunks 0/1 don't read them; the ramp-in steps
                    # are thin and would otherwise idle the PE)
                    if t == 0 and do_attn and NCH > 2 and cp[0] >= 2:
                        for which in ("q", "k", "v"):
                            projq.append((0, which, cp))
                    else:
                        emit_upfront(t, cp, warm=(t == 0 and cp[0] == 0))
            else:
                for which in ("q", "k", "v"):
                    for cp in cpairs:
                        projq.append((t, which, cp))

        for t in range(T):
            aT.append(singles.tile([PT, Sc], bf16, name=f"aT{t}"))
        wo_sb = pool_w.tile([PT, T, NT, PT], bf16, tag="w", name="wo_sb")
        nc.sync.dma_start(out=wo_sb[:, :, :, :], in_=wo_d.ap())

        # ---- phase 2: attention, one rolling software pipeline ----
        # A single S-stream (scores+mask+exp) runs LA steps ahead of the
        # A-stream (AV accumulation) across ALL (head-pair, chunk, key-tile)
        # steps, so the pipeline never drains at chunk or head-pair
        # boundaries and the PE never waits on the ACT engine's exp.
        LA = acc_bufs  # scores lookahead (steps) = pss slot count
        KC = CH // PT
        allsteps = [(t, c, kt)
                    for t in range(T if do_attn else 0)
                    for c in range(NCH)
                    for kt in range((c + 1) * KC)]
        pso_by_tc = {}
        nmw_by_t, dnw_by_t, lg_by_t, dm_by_t = {}, {}, {}, {}

        def emit_S(idx):
            t, c, kt = allsteps[idx]
            i = kt - c * KC            # >=0 on the diagonal superblock
            off = max(i, 0) * PT       # queries < off are fully masked
            if kt == 0:
                pso_by_tc[(t, c)] = {
                    h: ps_att.tile([PT, CH], f32, tag="pso", name=f"pso{t}{c}{h}")
                    for h in (0, 1)}
            pss = ps_acc.tile([PT, 2, CH], f32, tag="acc", name=f"ss{t}{c}{kt}")
            for h in (0, 1):
                hp = slice(h * DK, (h + 1) * DK)
                nc.tensor.matmul(
                    pss[:, h, off:CH],
                    kT_sb[t][hp, kt * PT:(kt + 1) * PT],
                    qT[t][hp, c * CH + off:(c + 1) * CH],
                    start=True, stop=True)
            if i >= 0:
                for h in (0, 1):
                    nc.vector.tensor_add(
                        pss[:, h, off:off + PT], pss[:, h, off:off + PT],
                        mask_sb[:, :])
            Pt = pool_P.tile([PT, 2, CH], bf16, tag="P", name=f"P{t}{c}{kt}")
            nc.scalar.activation(
                out=Pt[:, :, off:CH], in_=pss[:, :, off:CH], func=AF.Exp)
            return Pt, off

        def emit_A(idx, Pt, off):
            t, c, kt = allsteps[idx]
            nkt = (c + 1) * KC
            pso = pso_by_tc[(t, c)]
            for h in (0, 1):
                nc.tensor.matmul(
                    pso[h][:, off:CH],
                    v_sb[t][:, kt, h * PT:(h + 1) * PT],
                    Pt[:, h, off:CH],
                    start=(kt == 0), stop=(kt == nkt - 1),
                    skip_group_check=True)
            if kt == nkt - 1:
                # chunk done: evacuate numerators + denominators into the
                # per-head-pair wide tiles (copy-first frees the pso banks
                # immediately); the Ln/Exp/mul happen once per head-pair.
                if t not in nmw_by_t:
                    nmw_by_t[t] = pool_nmw.tile([PT, NCH, CH], f32, tag="nmw",
                                                name=f"nmw{t}")
                    dnw_by_t[t] = pool_dn.tile([PT, NCH, CH], f32, tag="dn",
                                               name=f"dnw{t}")
                nmw, dnw = nmw_by_t[t], dnw_by_t[t]
                for h in (0, 1):
                    hr = slice(h * DK, (h + 1) * DK)
                    nc.vector.tensor_copy(nmw[hr, c, :], pso[h][0:DK, :])
                    nc.vector.tensor_copy(dnw[hr, c, :], pso[h][DK:2 * DK, :])
                del pso_by_tc[(t, c)]
                last_of_t = (idx + 1 == len(allsteps)
                             or allsteps[idx + 1][0] != t)
                if t == T - 1:
                    # last head-pair: normalize per chunk so only the final
                    # chunk's small Ln/Exp sits between the last AV and the
                    # output projection's first ct=3 matmul
                    if t not in lg_by_t:
                        lg_by_t[t] = pool_dn.tile([PT, NCH, CH], f32,
                                                  tag="dn", name=f"lgw{t}")
                        dm_by_t[t] = pool_dmw.tile([PT, NCH, CH], bf16,
                                                   tag="dmw", name=f"dmw{t}")
                    lgw, dmw = lg_by_t[t], dm_by_t[t]
                    nc.scalar.activation(out=lgw[:, c, :], in_=dnw[:, c, :],
                                         func=AF.Ln)
                    nc.scalar.activation(out=dmw[:, c, :], in_=lgw[:, c, :],
                                         func=AF.Exp, scale=-1.0)
                    nc.vector.tensor_mul(
                        aT[t][:, c * CH:(c + 1) * CH],
                        nmw[:, c, :], dmw[:, c, :])
                    if last_of_t:
                        del nmw_by_t[t], dnw_by_t[t]
                    return
                if last_of_t:
                    # one batched Ln/Exp/mul per head-pair (the ACT engine
                    # is the attention bottleneck: 2 calls of N=2048 replace
                    # 8 calls of N=512)
                    lgw = pool_dn.tile([PT, NCH, CH], f32, tag="dn",
                                       name=f"lgw{t}")
                    # dmw in bf16: 1/denominator at 0.4% rel err is well
                    # within the gate and halves this pool's SBUF footprint
                    dmw = pool_dmw.tile([PT, NCH, CH], bf16, tag="dmw",
                                        name=f"dmw{t}")
                    nc.scalar.activation(out=lgw[:, :, :], in_=dnw[:, :, :],
                                         func=AF.Ln)
                    nc.scalar.activation(out=dmw[:, :, :], in_=lgw[:, :, :],
                                         func=AF.Exp, scale=-1.0)
                    nc.vector.tensor_mul(
                        aT[t][:, 0:NCH * CH].rearrange(
                            "p (c q) -> p c q", c=NCH),
                        nmw[:, :, :], dmw[:, :, :])
                if last_of_t:
                    del nmw_by_t[t], dnw_by_t[t]

        # pace the deferred projection sub-groups into the pipeline:
        # head-pair t's sub-groups are spread across the attention steps of
        # head-pair t-1, so each head-pair's projections finish (with
        # lookahead margin) before the attention steps that consume them
        first_step = {}
        for idx, (t_, _, _) in enumerate(allsteps):
            first_step.setdefault(t_, idx)
        gens_at = {}
        for tq in range(0, T):
            if not allsteps:
                break
            if tq == 0:
                # t0's deferred (2,3) groups must finish before the first
                # step that reads chunk 2 (start of chunk c=2), with margin
                lo = 0
                hi = max(allsteps.index((0, 2, 0)) - LA - 2, 0) \
                    if (0, 2, 0) in allsteps else 0
            else:
                lo = first_step[tq - 1]
                hi = max(first_step[tq] - LA - 2, lo)
            grp = [g for g in projq if g[0] == tq]
            if not grp:
                continue
            span = max(hi - lo, 1) / len(grp)
            for j, g in enumerate(grp):
                gen = proj_gen(*g)
                nsl = n_slices(g[1], g[2])
                for sl in range(nsl):
                    step = min(int(lo + span * j + span * (sl + 1) / nsl), hi)
                    gens_at.setdefault(step, []).append(gen)
        # during the last head-pair's attention there are no deferred
        # projections left and the window is ACT(exp)-bound: fill the free
        # "proj" PSUM slot with the first few output-projection groups'
        # ct=0..2 partials, spilled to SBUF as bf16 (phase 3 adds them back
        # during evacuation).
        spilled = {}

        def op_spill_gen(nt, cp):
            chunks = [c for c in cp if c < NCH]
            bt = ps_acc.tile([PT, 2, CH], f32, tag="proj",
                             name=f"osp{nt}{cp[0]}", bufs=1)
            banks = {c: bt[:, g, :] for g, c in enumerate(chunks)}
            for ct in range(T - 1):
                for c in chunks:
                    nc.tensor.matmul(
                        banks[c], wo_sb[:, ct, nt, :],
                        aT[ct][:, c * CH:(c + 1) * CH],
                        start=(ct == 0), stop=(ct == T - 2),
                        skip_group_check=True)
                yield
            sp = pool_sp.tile([PT, 2, CH], bf16, tag="sp",
                              name=f"sp{nt}{cp[0]}")
            nc.vector.tensor_copy(sp[:, 0:len(chunks), :],
                                  bt[:, 0:len(chunks), :])
            spilled[(nt, cp)] = sp
            yield

        # chunk-pair-major order: the (0,1)-chunk groups finish first and
        # only need aT3's chunks 0/1 (normalized mid-way through the last
        # head-pair's attention), so the pipeline never waits on the final
        # chunk's normalization chain at the attention->outproj boundary
        ogroups = ([(nt, cp) for cp in cpairs for nt in range(NT)]
                   if (do_attn and do_outproj) else [])
        spill_groups = ogroups[:4] if (do_attn and do_outproj and T > 1) else []
        spill_gens = []
        if spill_groups and allsteps:
            lo3 = first_step[T - 1] + 2
            hi3 = max(len(allsteps) - 2, lo3)
            span3 = max(hi3 - lo3, 1) / len(spill_groups)
            for j, g in enumerate(spill_groups):
                gen = op_spill_gen(*g)
                spill_gens.append(gen)
                for sl in range(T):
                    step = min(int(lo3 + span3 * j + span3 * (sl + 1) / T), hi3)
                    gens_at.setdefault(step, []).append(gen)

        inflight = {}
        for j in range(min(LA, len(allsteps))):
            inflight[j] = emit_S(j)
        for i in range(len(allsteps)):
            for gen in gens_at.get(i, []):
                next(gen, None)
            if i + LA < len(allsteps):
                inflight[i + LA] = emit_S(i + LA)
            emit_A(i, *inflight.pop(i))
        for gen in spill_gens:
            for _ in gen:
                pass

        # ---- phase 3: output projection (pipelined, partial, transposed) ----
        # (nt, chunk-pair) groups run a 2-deep software pipeline on the two
        # "acc" slots: group g's ct=0..2 accumulation runs while group g-1
        # finishes ct=3 (which waits on the last head-pair's normalization)
        # and evacuates, hiding that tail and spreading the output DMA.
        def op_start(nt, cp, tag):
            chunks = [c for c in cp if c < NCH]
            bt = ps_acc.tile([PT, 2, CH], f32, tag=tag, name=f"op{nt}{cp[0]}",
                             bufs=(1 if tag == "proj" else None))
            banks = {c: bt[:, g, :] for g, c in enumerate(chunks)}
            for ct in range(T - 1):
                for c in chunks:
                    nc.tensor.matmul(
                        banks[c], wo_sb[:, ct, nt, :],
                        aT[ct][:, c * CH:(c + 1) * CH],
                        start=(ct == 0), stop=False, skip_group_check=True)
            return banks

        def op_finish(nt, cp, banks, sp=None):
            chunks = [c for c in cp if c < NCH]
            for c in chunks:
                nc.tensor.matmul(
                    banks[c], wo_sb[:, T - 1, nt, :],
                    aT[T - 1][:, c * CH:(c + 1) * CH],
                    start=(sp is not None), stop=True, skip_group_check=True)
            ost = pool_os.tile([PT, 2, CH], bf16, tag="os")
            for g, c in enumerate(chunks):
                if sp is not None:
                    nc.vector.tensor_add(ost[:, g, :], banks[c], sp[:, g, :])
                else:
                    eng = (nc.scalar.copy if c % 2 == 0
                           else nc.vector.tensor_copy)
                    eng(ost[:, g, :], banks[c])
            nc.sync.dma_start(
                out=out_d.ap()[nt * PT:(nt + 1) * PT,
                               chunks[0] * CH:(chunks[-1] + 1) * CH],
                in_=ost[:, 0:len(chunks), :])

        # 3-deep pipeline: the "proj" slot is free once the attention-window
        # spills evacuated, so op-group tiles cycle acc/acc/proj.  Spilled
        # groups only need their ct=3 matmul (fresh accumulation) plus an
        # add-back of the bf16 partial during evacuation.
        pend = []
        for j, (nt, cp) in enumerate(ogroups):
            sp = spilled.get((nt, cp))
            if sp is not None:
                bt = ps_acc.tile([PT, 2, CH], f32, tag="acc",
                                 name=f"opf{nt}{cp[0]}")
                chunks = [c for c in cp if c < NCH]
                banks = {c: bt[:, g, :] for g, c in enumerate(chunks)}
                pend.append((nt, cp, banks, sp))
            else:
                banks = op_start(nt, cp, "proj" if j % 3 == 2 else "acc")
                pend.append((nt, cp, banks, None))
            if len(pend) > 2:
                op_finish(*pend.pop(0))
        for p in pend:
            op_finish(*p)

    _dedup_ldweights(nc)
    if split_waits:
        _split_multi_waits(nc)
    return nc


_build_cache = {}


def _get_program(Sc=S):
    key = Sc
    if key not in _build_cache:
        _build_cache[key] = build(Sc)
    return _build_cache[key]


def _bf16(a):
    import ml_dtypes
    return np.ascontiguousarray(a).astype(ml_dtypes.bfloat16)


def make_in_maps(x, w_q, w_k, w_v, w_o):
    """Host-side sharding: returns per-core input dicts.
    Core c: batch c//2, head-half c%2."""
    Bc, Sc, Dc = x.shape
    scale = DK ** -0.5
    ident = np.eye(PT, dtype=np.float32)
    jj, qq = np.meshgrid(np.arange(PT), np.arange(PT), indexing="ij")
    mask = np.where(jj <= qq, 0.0, NEG).astype(np.float32)

    def pack_w(w):  # [1024, 512] -> [128 p, 8 kt, 4 t, 128 c]
        return np.ascontiguousarray(
            w.reshape(KT, PT, T, PT).transpose(1, 0, 2, 3))

    xTs = [_bf16(x[b].T.reshape(KT, PT, Sc)) for b in range(Bc)]
    whalf = []
    for hh in range(2):
        rows = slice(512 * hh, 512 * hh + 512)
        wo_half = w_o[:, rows].T.reshape(T, PT, NT, PT).transpose(1, 0, 2, 3)
        whalf.append({
            "wq": _bf16(pack_w((w_q[rows, :] * scale).T)),
            "wk": _bf16(pack_w(w_k[rows, :].T)),
            "wv": _bf16(pack_w(w_v[rows, :].T)),
            "wo": _bf16(np.ascontiguousarray(wo_half)),
        })
    in_maps = []
    for c in range(NCORES):
        b, hh = c // 2, c % 2
        m = {"xT": xTs[b], "ident": _bf16(ident), "mask": mask}
        m.update(whalf[hh])
        in_maps.append(m)
    return in_maps


def run_on_hw(in_maps, Sc=S, trace=False, trace_cores=None):
    nc = _get_program(Sc)
    return bass_utils.run_bass_kernel_spmd(
        nc, in_maps, core_ids=list(range(NCORES)), trace=trace,
        trace_cores=trace_cores)


def kernel(x, w_q, w_k, w_v, w_o, b_o):
    x = np.asarray(x, dtype=np.float32)
    w_q = np.asarray(w_q, dtype=np.float32)
    w_k = np.asarray(w_k, dtype=np.float32)
    w_v = np.asarray(w_v, dtype=np.float32)
    w_o = np.asarray(w_o, dtype=np.float32)
    b_o = np.asarray(b_o, dtype=np.float32)
    Bc, Sc, Dc = x.shape
    in_maps = make_in_maps(x, w_q, w_k, w_v, w_o)
    res = run_on_hw(in_maps, Sc)
    out = np.empty((Bc, Sc, Dc), dtype=np.float32)
    for b in range(Bc):
        outT = (np.asarray(res.results[2 * b]["outT"], dtype=np.float32)
                + np.asarray(res.results[2 * b + 1]["outT"], dtype=np.float32))
        out[b] = outT.T + b_o
    return out

